# revision 34
# baseline (speedup 1.0000x reference)
"""LowRankGlobalAttention TRN2 Bass kernel (8-core SPMD).

out = concat(relu(xW+b)[:, :32] @ (V^T Z) * D, T) where
U,V,Z,T = relu(xW+b) column blocks, D = 1/(sum(U @ colsum(V))/N + eps).

Per core (row-sharded, 62500 rows each), phase 1 streams x once
(serialized-DMA floor ~1.4 us per 512-row supertile) through a
software pipeline with no same-iteration cross-engine round trips:
  transp:  PE 8x [128,128] f32r transposes (x dtype-punned to f32r:
           1.5 c/row instead of f32's 2.0)
  xtcopy:  DVE+ACT split-copy x^T PSUM->SBUF f32r (GEMM rhs must be
           SBUF)
  gemm:    PE W-stationary f32r GEMM -> X^T in PSUM. W column order
           alternates per supertile parity ([U,T,V,Z] even,
           [V,Z,U,T] odd) so the [U^T;T^T] half lands at the slab
           base partition (0/64) its persistent store expects.
  relu:    one ACT relu -> X^T bf16 rotating tile, accum_out ->
           per-parity colsum accumulators (GpSimd adds; the parity
           flip keeps U/T vs V/Z halves at fixed partition rows)
  park:    DVE copies the [U^T;T^T] half into its persistent slab
           (bf16 2x mode; no partition shift thanks to the parity
           flip, replacing the baseline's 1.9us GpSimd park)
  stageC:  PE bf16 transposes -> [n, V|Z] rows in PSUM
  vtz:     PE bf16 matmuls accumulate VtZ in a PSUM bank.
stageC and VtZ run batched over supertile PAIRS on even iterations:
the PE pays a pipeline-drain tax (~100-280ns) every time the group
dtype/ISA mode changes, so fewer, bigger bf16 groups are cheaper.
Stats (VtZ [32,32], colsum(V), colsum(U)) AllReduce across 8 cores;
D and M = D*VtZ (bf16, replicated at bases 0/64) computed on-chip.
Phase 2 per 512-row tile: PE transposes T^T slabs back to rows
(bf16 1.0 c/row), res = U @ M with bf16 U^T-stationary (1.0 c/row vs
f32r's 4.0 small-matmul penalty), DVE/ACT interleave [res|T] f32,
one DMA per tile alternating the SP/ACT HWDGE rings. T-prep runs 2
tiles ahead of res, and NPREP tiles prestage during the AllReduce.
"""
import numpy as np

import concourse.bass as bass
import concourse.mybir as mybir
import concourse.tile as tile
from concourse import bacc
from concourse.bass_utils import run_bass_kernel_spmd
from concourse.masks import make_identity

F32 = mybir.dt.float32
F32R = mybir.dt.float32r
BF16 = mybir.dt.bfloat16

N_CORES = 8
N_TOTAL = 500000
NR = N_TOTAL // N_CORES          # 62500 rows per core
D_IN = 256
KATT = 32
R = 512                          # supertile rows
G = 4                            # row-blocks per supertile (R/128)
NS = NR // R                     # 122 full supertiles
NP = NS // 2                     # 61 x pair-DMAs (2 supertiles each)
TAIL = NR - NS * R               # 36 tail rows
NU = (NS + 1 + 1) // 2           # [U.T;T.T] slab tiles (2 per tile)
NPREP = 52                       # out tiles T-prepped in the AR window
XSPL = 592                       # x^T copy free-elems done by DVE (rest ACT)
EPS = 1e-6

# even supertiles: W columns [U | T | V | Z]; odd: [V | Z | U | T]
PERM_E = np.concatenate([np.arange(0, 32), np.arange(96, 128),
                         np.arange(32, 64), np.arange(64, 96)])
PERM_O = np.concatenate([PERM_E[64:], PERM_E[:64]])

_CACHE = {}


def _build():
    nc = bacc.Bacc(None)
    # f32r so the PE transpose runs at 1.5 c/row (np dtype is still
    # float32; the bits are identical)
    x_in = nc.dram_tensor("xin", [NR, D_IN], F32R, kind="ExternalInput")
    we_in = nc.dram_tensor("we", [D_IN, 128], F32, kind="ExternalInput")
    wo_in = nc.dram_tensor("wo", [D_IN, 128], F32, kind="ExternalInput")
    be_in = nc.dram_tensor("be", [128], F32, kind="ExternalInput")
    bo_in = nc.dram_tensor("bo", [128], F32, kind="ExternalInput")
    # tail-pad corrections: [32,32] = n_pad*rbV x rbZ, [128] =
    # n_pad*relu(b_even) for the even-parity colsum accumulator
    cvz_in = nc.dram_tensor("corrvz", [KATT, KATT], F32,
                            kind="ExternalInput")
    cut_in = nc.dram_tensor("corrcs", [128], F32, kind="ExternalInput")
    out_d = nc.dram_tensor("out", [NR, 2 * KATT], F32, kind="ExternalOutput")

    stats_in = nc.dram_tensor("stats_in", [1088], F32)
    stats_out = nc.dram_tensor("stats_out", [1088], F32, addr_space="Shared")

    with tile.TileContext(nc) as tc:
        with tc.tile_pool(name="const", bufs=1) as const, \
             tc.tile_pool(name="ustore", bufs=1) as ustore, \
             tc.tile_pool(name="small", bufs=1) as small, \
             tc.tile_pool(name="vtzps", bufs=1, space="PSUM") as vtzps:

            # ---- constants ----
            w_st = const.tile([128, 2, 2, 128], F32, tag="wst")
            nc.sync.dma_start(w_st[:, 0], we_in.ap().rearrange(
                "(g p) c -> p g c", g=2))
            nc.sync.dma_start(w_st[:, 1], wo_in.ap().rearrange(
                "(g p) c -> p g c", g=2))
            w_r = const.tile([128, 2, 2, 128], BF16, tag="wr")
            nc.vector.tensor_copy(w_r[:], w_st[:])
            b_sb = const.tile([128, 2], F32, tag="bsb")
            nc.sync.dma_start(b_sb[:, 0:1],
                              be_in.ap().rearrange("(p o) -> p o", o=1))
            nc.sync.dma_start(b_sb[:, 1:2],
                              bo_in.ap().rearrange("(p o) -> p o", o=1))
            ident = const.tile([128, 128], F32, tag="ident")
            make_identity(nc, ident[:])
            identr = const.tile([128, 128], F32R, tag="identr")
            nc.vector.tensor_copy(identr[:], ident[:])
            identb = const.tile([128, 128], BF16, tag="identb")
            nc.vector.tensor_copy(identb[:], ident[:])
            ones1 = const.tile([1, 128], F32, tag="ones1")
            nc.gpsimd.memset(ones1[:], 1.0)
            csum_e = const.tile([128, 1], F32, tag="csume")
            nc.gpsimd.memset(csum_e[:], 0.0)
            csum_o = const.tile([128, 1], F32, tag="csumo")
            nc.gpsimd.memset(csum_o[:], 0.0)
            cvz_sb = const.tile([KATT, KATT], F32, tag="cvz")
            nc.sync.dma_start(cvz_sb[:], cvz_in[:, :])
            cut_sb = const.tile([128, 1], F32, tag="cut")
            nc.sync.dma_start(cut_sb[:],
                              cut_in.ap().rearrange("(p o) -> p o", o=1))

            # persistent [U.T;T.T] bf16 slabs, 2 supertiles per tile at
            # base partitions 0/64 (PE stationary base constraint)
            st_tiles = [ustore.tile([128, R], BF16, tag=f"u{j}", name=f"u{j}")
                        for j in range(NU)]

            def slab(s):
                return st_tiles[s // 2], 64 * (s % 2)

            vtz_ps = vtzps.tile([KATT, KATT], F32, tag="vtz")

            # ---------------- phase 1 ----------------
            with tc.tile_pool(name="p1x", bufs=5) as p1x, \
                 tc.tile_pool(name="p1xt", bufs=3) as p1xt, \
                 tc.tile_pool(name="p1vzf", bufs=4) as p1vzf, \
                 tc.tile_pool(name="p1vz", bufs=3) as p1vz, \
                 tc.tile_pool(name="p1cs", bufs=4) as p1cs, \
                 tc.tile_pool(name="xtps", bufs=2, space="PSUM") as xtps, \
                 tc.tile_pool(name="mmps", bufs=2, space="PSUM") as mmps, \
                 tc.tile_pool(name="scps", bufs=1, space="PSUM") as scps:

                vz_tiles = [p1vz.tile([128, 2, G, 2 * KATT], BF16,
                                      tag="vzsb", name=f"vzsb{j}")
                            for j in range(3)]

                st_pair = {}
                st_xtps = {}    # transp out PSUM, stage 0 -> -1
                st_xtsb = {}    # x^T SBUF, stage -1 -> -2
                st_mm = {}      # GEMM out PSUM, stage -2 -> -3
                st_relu = {}    # X^T bf16, stage -3 -> -4/-5 (park+stageC)
                st_scout = {}   # stageC PSUM (tile, q), even iters
                st_vzrows = {}  # V|Z rows SBUF (tile, q)
                n_vtz = 0

                # ---- steady-state pipeline ----
                for p in range(2):
                    xp = p1x.tile([128, 2, G, D_IN], F32R, tag="xin")
                    eng = nc.sync if p % 2 == 0 else nc.gpsimd
                    eng.dma_start(
                        xp[:],
                        x_in[2 * p * R:(2 * p + 2) * R, :].rearrange(
                            "(t p g) d -> p t g d", t=2, g=G))
                    st_pair[p] = xp

                # ---- tail first (36 rows zero-padded, even layout) so
                # its serial chain hides in the pipeline fill and its
                # VtZ opens the accumulation group ----
                x_tl = p1x.tile([128, 2, G, D_IN], F32R, tag="xin")
                zpad = small.tile([128, D_IN], F32, tag="zpad", name="zpad")
                nc.gpsimd.memset(zpad[:], 0.0)
                nc.vector.tensor_copy(x_tl[:, 0, 0], zpad[:])
                nc.sync.dma_start(x_tl[0:TAIL, 0, 0, :],
                                  x_in[NS * R:NR, :])
                xt_ps = xtps.tile([128, 2, R], F32R, tag="xt")
                for k in range(2):
                    nc.tensor.transpose(
                        xt_ps[:, k, 0:128],
                        x_tl[:, 0, 0, k * 128:(k + 1) * 128], identr[:])
                xt_sb = p1xt.tile([128, 2, R], BF16, tag="xtsb")
                nc.vector.tensor_copy(xt_sb[:, :, 0:128], xt_ps[:, :, 0:128])
                mm_ps = mmps.tile([128, R], F32, tag="xmm")
                nc.tensor.matmul(mm_ps[:, 0:128], w_r[:, 0, 0],
                                 xt_sb[:, 0, 0:128], start=True, stop=False)
                nc.tensor.matmul(mm_ps[:, 0:128], w_r[:, 0, 1],
                                 xt_sb[:, 1, 0:128], start=False, stop=True)
                ut_t, _ = slab(NS)
                cs_t = p1cs.tile([128, 1], F32, tag="cst")
                vzf_t = p1vzf.tile([128, R], BF16, tag="vzf")
                nc.scalar.activation(vzf_t[:, 0:128], mm_ps[:, 0:128],
                                     mybir.ActivationFunctionType.Relu,
                                     bias=b_sb[:, 0:1],
                                     accum_out=cs_t[:])
                nc.gpsimd.tensor_add(csum_e[:], csum_e[:], cs_t[:])
                nc.vector.tensor_copy(ut_t[0:64, 0:128], vzf_t[0:64, 0:128])
                sc_tl = scps.tile([128, 2, G, 2 * KATT], BF16, tag="sc")
                nc.tensor.transpose(sc_tl[:, 0, 0, :], vzf_t[64:128, 0:128],
                                    identb[64:128, 64:128])
                vz_tl = vz_tiles[2]
                nc.vector.tensor_copy(vz_tl[:, 0, 0, 0:2 * KATT],
                                      sc_tl[:, 0, 0, :])
                nc.tensor.matmul(vtz_ps[:], vz_tl[:, 0, 0, 0:KATT],
                                 vz_tl[:, 0, 0, KATT:2 * KATT],
                                 start=True, stop=False)
                n_vtz += 1

                for i in range(NS + 9):
                    # oldest PE work first so a late x DMA can't block
                    # ready matmuls in the in-order PE queue
                    if True:
                        for s in (i - 7,):
                            if not 0 <= s < NS:
                                continue
                            vzb, q = st_vzrows.pop(s)
                            for c in range(G):
                                nc.tensor.matmul(
                                    vtz_ps[:], vzb[:, q, c, 0:KATT],
                                    vzb[:, q, c, KATT:2 * KATT],
                                    start=False,
                                    stop=(n_vtz == NS) and (c == G - 1))
                            n_vtz += 1
                        sc_ps = None
                        for q, s in enumerate((i - 4,)):
                            if not 0 <= s < NS:
                                continue
                            if sc_ps is None:
                                sc_ps = scps.tile([128, 2, G, 2 * KATT],
                                                  BF16, tag="sc")
                            vzf = st_relu[s]
                            lo = 64 * ((s + 1) % 2)
                            for c in range(G):
                                nc.tensor.transpose(
                                    sc_ps[:, q, c, :],
                                    vzf[lo:lo + 64, c * 128:(c + 1) * 128],
                                    identb[lo:lo + 64, lo:lo + 64])
                            st_scout[s] = (sc_ps, q)

                    if 0 <= i - 2 < NS:
                        s = i - 2
                        xt_sb = st_xtsb.pop(s)
                        par = s % 2
                        mm_ps = mmps.tile([128, R], F32, tag="xmm")
                        nc.tensor.matmul(mm_ps[:], w_r[:, par, 0],
                                         xt_sb[:, 0, :], start=True,
                                         stop=False)
                        nc.tensor.matmul(mm_ps[:], w_r[:, par, 1],
                                         xt_sb[:, 1, :], start=False,
                                         stop=True)
                        st_mm[s] = mm_ps

                    if i < NS:
                        s = i
                        if s % 2 == 0:
                            p = s // 2 + 2
                            if p < NP:
                                xp = p1x.tile([128, 2, G, D_IN], F32R,
                                              tag="xin")
                                eng = nc.sync if p % 2 == 0 else nc.gpsimd
                                eng.dma_start(
                                    xp[:],
                                    x_in[2 * p * R:(2 * p + 2) * R, :]
                                    .rearrange("(t p g) d -> p t g d",
                                               t=2, g=G))
                                st_pair[p] = xp
                        xp = st_pair[s // 2]
                        xt_ps = xtps.tile([128, 2, R], F32R, tag="xt")
                        for k in range(2):
                            for g in range(G):
                                nc.tensor.transpose(
                                    xt_ps[:, k, g * 128:(g + 1) * 128],
                                    xp[:, s % 2, g,
                                       k * 128:(k + 1) * 128],
                                    identr[:])
                        st_xtps[s] = xt_ps
                        if s % 2 == 1:
                            st_pair.pop(s // 2)

                    # DVE: x^T copy first (its consumer, the GEMM, is
                    # the tightest dependency)
                    if 0 <= i - 1 < NS:
                        s = i - 1
                        xt_ps = st_xtps.pop(s)
                        xt_sb = p1xt.tile([128, 2, R], BF16, tag="xtsb")
                        h = XSPL // 2
                        nc.vector.tensor_copy(xt_sb[:, :, 0:h],
                                              xt_ps[:, :, 0:h])
                        nc.scalar.copy(xt_sb[:, :, h:R], xt_ps[:, :, h:R])
                        st_xtsb[s] = xt_sb

                    if True:
                        for s in (i - 5,):
                            if s in st_scout:
                                sc_ps, q = st_scout.pop(s)
                                vzb = vz_tiles[s % 3]
                                nc.vector.tensor_copy(vzb[:, q],
                                                      sc_ps[:, q])
                                st_vzrows[s] = (vzb, q)

                    if 0 <= i - 4 < NS:
                        s = i - 4
                        vzf = st_relu[s]
                        uo = 64 * (s % 2)
                        ut_t, _ = slab(s)
                        nc.vector.tensor_copy(ut_t[uo:uo + 64, :],
                                              vzf[uo:uo + 64, :])
                    st_relu.pop(i - 6, None)

                    if 0 <= i - 3 < NS:
                        s = i - 3
                        mm_ps = st_mm.pop(s)
                        par = s % 2
                        vzf = p1vzf.tile([128, R], BF16, tag="vzf")
                        cs_t = p1cs.tile([128, 1], F32, tag="cst")
                        nc.scalar.activation(
                            vzf[:], mm_ps[:],
                            mybir.ActivationFunctionType.Relu,
                            bias=b_sb[:, par:par + 1],
                            accum_out=cs_t[:])
                        csacc = csum_e if par == 0 else csum_o
                        nc.gpsimd.tensor_add(csacc[:], csacc[:], cs_t[:])
                        st_relu[s] = vzf

            # ---------------- phase 2 pools + T prep ----------------
            with tc.tile_pool(name="p2o", bufs=NPREP + 6) as p2o, \
                 tc.tile_pool(name="p2ps", bufs=3, space="PSUM") as p2ps, \
                 tc.tile_pool(name="p2tps", bufs=2, space="PSUM") as p2tps:

                def t_prep(s, ot, eng):
                    # T rows for supertile s -> ot[:, :, 32:64]. The full
                    # 64-row [U^T;T^T] slab is transposed so the PE
                    # stationary base stays 0/64; U half unused.
                    t2 = p2tps.tile([128, G, 2 * KATT], BF16, tag="t2")
                    tt, to = slab(s)
                    for c in range(G):
                        nc.tensor.transpose(
                            t2[:, c, :],
                            tt[to:to + 64, c * 128:(c + 1) * 128],
                            identb[to:to + 64, to:to + 64])
                    if eng == 0:
                        nc.vector.tensor_copy(ot[:, :, KATT:2 * KATT],
                                              t2[:, :, KATT:2 * KATT])
                    else:
                        nc.scalar.copy(ot[:, :, KATT:2 * KATT],
                                       t2[:, :, KATT:2 * KATT])

                ots = []
                for s in range(min(NPREP, NS)):
                    ot = p2o.tile([128, G, 2 * KATT], F32, tag="oo")
                    t_prep(s, ot, s % 2)
                    ots.append(ot)

                # ---------------- stats + collective ----------------
                vtz_sb = small.tile([KATT, KATT], F32, tag="vtzsb")
                nc.vector.tensor_sub(vtz_sb[:], vtz_ps[:], cvz_sb[:])
                nc.vector.tensor_sub(csum_e[:], csum_e[:], cut_sb[:])
                nc.sync.dma_start(
                    stats_in[0:1024].rearrange("(p q) -> p q", p=KATT),
                    vtz_sb[:])
                # colsum(V) = even rows 64:96 + odd rows 0:32;
                # colsum(U) = even rows 0:32 + odd rows 64:96
                cs_fold = small.tile([KATT, 2], F32, tag="csfold")
                nc.gpsimd.tensor_copy(cs_fold[:, 0:1], csum_e[64:96, :])
                nc.gpsimd.tensor_copy(cs_fold[:, 1:2], csum_o[64:96, :])
                nc.vector.tensor_add(cs_fold[:, 0:1], cs_fold[:, 0:1],
                                     csum_o[0:32, :])
                nc.vector.tensor_add(cs_fold[:, 1:2], cs_fold[:, 1:2],
                                     csum_e[0:32, :])
                nc.sync.dma_start(
                    stats_in[1024:1088].rearrange("(q p) -> p q", q=2),
                    cs_fold[:])
                nc.gpsimd.collective_compute(
                    "AllReduce", mybir.AluOpType.add,
                    replica_groups=[list(range(N_CORES))],
                    ins=[stats_in.ap().opt()], outs=[stats_out.ap().opt()])
                us_all = small.tile([KATT, 2], F32, tag="usall")
                nc.sync.dma_start(
                    us_all[:],
                    stats_out[1024:1088].rearrange("(q p) -> p q", q=2))
                vtz_g = small.tile([128, KATT], F32, tag="vtzg")
                for q in range(2):
                    nc.sync.dma_start(
                        vtz_g[64 * q:64 * q + KATT, :],
                        stats_out[0:1024].rearrange("(p q) -> p q", p=KATT))

                with tc.tile_pool(name="dps", bufs=1, space="PSUM") as dps:
                    us_ps = dps.tile([1, 1], F32, tag="us")
                    nc.tensor.matmul(us_ps[:], us_all[:, 1:2],
                                     us_all[:, 0:1], start=True, stop=True)
                    nf_sb = small.tile([1, 1], F32, tag="nf")
                    nc.scalar.activation(nf_sb[:], us_ps[:],
                                         mybir.ActivationFunctionType.Copy,
                                         bias=EPS, scale=1.0 / N_TOTAL)
                    d_sb = small.tile([1, 1], F32, tag="dsb")
                    nc.vector.reciprocal(d_sb[:], nf_sb[:])
                    d_ps = dps.tile([128, 1], F32, tag="dps")
                    nc.tensor.matmul(d_ps[:], ones1[:], d_sb[:],
                                     start=True, stop=True)
                    d_all = small.tile([128, 1], F32, tag="dall")
                    nc.vector.tensor_copy(d_all[:], d_ps[:])
                vtz_d = small.tile([128, KATT], F32, tag="vtzd")
                m_all = small.tile([128, KATT], BF16, tag="mall")
                for q in (0, 64):
                    nc.vector.tensor_scalar_mul(vtz_d[q:q + KATT, :],
                                                vtz_g[q:q + KATT, :],
                                                d_all[q:q + KATT, :])
                    nc.vector.tensor_copy(m_all[q:q + KATT, :],
                                          vtz_d[q:q + KATT, :])

                # tail (full-width ops on the zero-padded block)
                ot = p2o.tile([128, G, 2 * KATT], F32, tag="oo")
                t2 = p2tps.tile([128, G, 2 * KATT], BF16, tag="t2")
                tt, to = slab(NS)
                nc.tensor.transpose(t2[:, 0, :], tt[to:to + 64, 0:128],
                                    identb[to:to + 64, to:to + 64])
                nc.scalar.copy(ot[:, 0, KATT:2 * KATT],
                               t2[:, 0, KATT:2 * KATT])
                res_ps = p2ps.tile([128, G, KATT], F32, tag="res")
                nc.tensor.matmul(res_ps[:, 0, :],
                                 tt[to:to + 32, 0:128],
                                 m_all[to:to + 32, :], start=True, stop=True)
                nc.vector.tensor_copy(ot[:, 0, 0:KATT], res_ps[:, 0, :])
                nc.sync.dma_start(out_d[NS * R:NR, :], ot[0:TAIL, 0, :])

                # T-prep runs 2 tiles ahead of res so its PE transposes
                # and PSUM->SBUF copies never stall the res/DMA chain
                ot_by_s = dict(enumerate(ots))
                for s in range(NS):
                    sp = s + 3
                    if len(ots) <= sp < NS:
                        otp = p2o.tile([128, G, 2 * KATT], F32, tag="oo")
                        t_prep(sp, otp, 0 if sp % 3 != 0 else 1)
                        ot_by_s[sp] = otp
                    ot = ot_by_s.pop(s)
                    res_ps = p2ps.tile([128, G, KATT], F32, tag="res")
                    ut_t, uo = slab(s)
                    for c in range(G):
                        nc.tensor.matmul(
                            res_ps[:, c, :],
                            ut_t[uo:uo + 32, c * 128:(c + 1) * 128],
                            m_all[uo:uo + 32, :],
                            start=True, stop=True)
                    nc.vector.tensor_copy(ot[:, :, 0:KATT], res_ps[:])
                    dma_eng = (nc.sync, nc.scalar, nc.gpsimd)[s % 3]
                    dma_eng.dma_start(
                        out_d[s * R:(s + 1) * R, :].rearrange(
                            "(p g) q -> p g q", g=G), ot[:])
    nc.compile()
    return nc


def _prep_inputs(x, W, b):
    W = np.asarray(W, dtype=np.float32)
    b = np.asarray(b, dtype=np.float32)
    we = np.ascontiguousarray(W[:, PERM_E])
    wo = np.ascontiguousarray(W[:, PERM_O])
    be = np.ascontiguousarray(b[PERM_E])
    bo = np.ascontiguousarray(b[PERM_O])
    rb = np.maximum(be, 0.0).astype(np.float32)
    n_pad = 128 - TAIL
    # tail pollution: relu(b) in the padded rows (even layout)
    rbv, rbz = rb[64:96], rb[96:128]
    corrvz = (n_pad * np.outer(rbv, rbz)).astype(np.float32)
    corrcs = (n_pad * rb).astype(np.float32)
    x = np.asarray(x, dtype=np.float32)
    in_maps = []
    for c in range(N_CORES):
        in_maps.append({
            "xin": np.ascontiguousarray(x[c * NR:(c + 1) * NR]),
            "we": we, "wo": wo, "be": be, "bo": bo,
            "corrvz": corrvz, "corrcs": corrcs,
        })
    return in_maps


def _run(x, W, b, trace=False):
    if "nc" not in _CACHE:
        _CACHE["nc"] = _build()
    nc = _CACHE["nc"]
    in_maps = _prep_inputs(x, W, b)
    res = run_bass_kernel_spmd(nc, in_maps, core_ids=list(range(N_CORES)),
                               trace=trace)
    out = np.concatenate([r["out"] for r in res.results], axis=0)
    return out, res


def kernel(x, W, b):
    out, _ = _run(x, W, b, trace=False)
    return out


# revision 36
# speedup vs baseline: 1.1862x; 1.1862x over previous
"""LowRankGlobalAttention TRN2 Bass kernel (8-core SPMD).

out = concat(relu(xW+b)[:, :32] @ (V^T Z) * D, T) where
U,V,Z,T = relu(xW+b) column blocks, D = 1/(sum(U @ colsum(V))/N + eps).

Per core (row-sharded, 62500 rows each), phase 1 streams x once
(serialized-DMA floor ~1.4 us per 512-row supertile) through a
software pipeline with no same-iteration cross-engine round trips:
  transp:  PE 8x [128,128] f32r transposes (x dtype-punned to f32r:
           1.5 c/row instead of f32's 2.0)
  xtcopy:  DVE+ACT split-copy x^T PSUM->SBUF f32r (GEMM rhs must be
           SBUF)
  gemm:    PE W-stationary f32r GEMM -> X^T in PSUM. W column order
           alternates per supertile parity ([U,T,V,Z] even,
           [V,Z,U,T] odd) so the [U^T;T^T] half lands at the slab
           base partition (0/64) its persistent store expects.
  relu:    one ACT relu -> X^T bf16 rotating tile, accum_out ->
           per-parity colsum accumulators (GpSimd adds; the parity
           flip keeps U/T vs V/Z halves at fixed partition rows)
  park:    DVE copies the [U^T;T^T] half into its persistent slab
           (bf16 2x mode; no partition shift thanks to the parity
           flip, replacing the baseline's 1.9us GpSimd park)
  stageC:  PE bf16 transposes -> [n, V|Z] rows in PSUM
  vtz:     PE bf16 matmuls accumulate VtZ in a PSUM bank.
stageC and VtZ run batched over supertile PAIRS on even iterations:
the PE pays a pipeline-drain tax (~100-280ns) every time the group
dtype/ISA mode changes, so fewer, bigger bf16 groups are cheaper.
Stats (VtZ [32,32], colsum(V), colsum(U)) AllReduce across 8 cores;
D and M = D*VtZ (bf16, replicated at bases 0/64) computed on-chip.
Phase 2 per 512-row tile: PE transposes T^T slabs back to rows
(bf16 1.0 c/row), res = U @ M with bf16 U^T-stationary (1.0 c/row vs
f32r's 4.0 small-matmul penalty), DVE/ACT interleave [res|T] f32,
one DMA per tile alternating the SP/ACT HWDGE rings. T-prep runs 2
tiles ahead of res, and NPREP tiles prestage during the AllReduce.
"""
import numpy as np

import concourse.bass as bass
import concourse.mybir as mybir
import concourse.tile as tile
from concourse import bacc
from concourse.bass_utils import run_bass_kernel_spmd
from concourse.masks import make_identity

F32 = mybir.dt.float32
F32R = mybir.dt.float32r
BF16 = mybir.dt.bfloat16

N_CORES = 8
N_TOTAL = 500000
NR = N_TOTAL // N_CORES          # 62500 rows per core
D_IN = 256
KATT = 32
R = 512                          # supertile rows
G = 4                            # row-blocks per supertile (R/128)
NS = NR // R                     # 122 full supertiles
NP = NS // 2                     # 61 x pair-DMAs (2 supertiles each)
TAIL = NR - NS * R               # 36 tail rows
NU = (NS + 1 + 1) // 2           # [U.T;T.T] slab tiles (2 per tile)
NPREP = 40                       # out tiles T-prepped in the AR window
XSPL = 592                       # x^T copy free-elems done by DVE (rest ACT)
EPS = 1e-6

# even supertiles: W columns [U | T | V | Z]; odd: [V | Z | U | T]
PERM_E = np.concatenate([np.arange(0, 32), np.arange(96, 128),
                         np.arange(32, 64), np.arange(64, 96)])
PERM_O = np.concatenate([PERM_E[64:], PERM_E[:64]])

_CACHE = {}


def _build():
    nc = bacc.Bacc(None)
    # f32r so the PE transpose runs at 1.5 c/row (np dtype is still
    # float32; the bits are identical)
    x_in = nc.dram_tensor("xin", [NR, D_IN], F32R, kind="ExternalInput")
    we_in = nc.dram_tensor("we", [D_IN, 128], F32, kind="ExternalInput")
    wo_in = nc.dram_tensor("wo", [D_IN, 128], F32, kind="ExternalInput")
    be_in = nc.dram_tensor("be", [128], F32, kind="ExternalInput")
    bo_in = nc.dram_tensor("bo", [128], F32, kind="ExternalInput")
    # tail-pad corrections: [32,32] = n_pad*rbV x rbZ, [128] =
    # n_pad*relu(b_even) for the even-parity colsum accumulator
    cvz_in = nc.dram_tensor("corrvz", [KATT, KATT], F32,
                            kind="ExternalInput")
    cut_in = nc.dram_tensor("corrcs", [128], F32, kind="ExternalInput")
    out_d = nc.dram_tensor("out", [NR, 2 * KATT], F32, kind="ExternalOutput")

    stats_in = nc.dram_tensor("stats_in", [1088], F32)
    stats_out = nc.dram_tensor("stats_out", [1088], F32, addr_space="Shared")

    with tile.TileContext(nc) as tc:
        with tc.tile_pool(name="const", bufs=1) as const, \
             tc.tile_pool(name="ustore", bufs=1) as ustore, \
             tc.tile_pool(name="small", bufs=1) as small, \
             tc.tile_pool(name="vtzps", bufs=1, space="PSUM") as vtzps:

            # ---- constants ----
            w_st = const.tile([128, 2, 2, 128], F32, tag="wst")
            nc.sync.dma_start(w_st[:, 0], we_in.ap().rearrange(
                "(g p) c -> p g c", g=2))
            nc.sync.dma_start(w_st[:, 1], wo_in.ap().rearrange(
                "(g p) c -> p g c", g=2))
            w_r = const.tile([128, 2, 2, 128], BF16, tag="wr")
            nc.vector.tensor_copy(w_r[:], w_st[:])
            b_sb = const.tile([128, 2], F32, tag="bsb")
            nc.sync.dma_start(b_sb[:, 0:1],
                              be_in.ap().rearrange("(p o) -> p o", o=1))
            nc.sync.dma_start(b_sb[:, 1:2],
                              bo_in.ap().rearrange("(p o) -> p o", o=1))
            ident = const.tile([128, 128], F32, tag="ident")
            make_identity(nc, ident[:])
            identr = const.tile([128, 128], F32R, tag="identr")
            nc.vector.tensor_copy(identr[:], ident[:])
            identb = const.tile([128, 128], BF16, tag="identb")
            nc.vector.tensor_copy(identb[:], ident[:])
            ones1 = const.tile([1, 128], F32, tag="ones1")
            nc.gpsimd.memset(ones1[:], 1.0)
            csum_e = const.tile([128, 1], F32, tag="csume")
            nc.gpsimd.memset(csum_e[:], 0.0)
            csum_o = const.tile([128, 1], F32, tag="csumo")
            nc.gpsimd.memset(csum_o[:], 0.0)
            cvz_sb = const.tile([KATT, KATT], F32, tag="cvz")
            nc.sync.dma_start(cvz_sb[:], cvz_in[:, :])
            cut_sb = const.tile([128, 1], F32, tag="cut")
            nc.sync.dma_start(cut_sb[:],
                              cut_in.ap().rearrange("(p o) -> p o", o=1))

            # persistent [U.T;T.T] bf16 slabs, 2 supertiles per tile at
            # base partitions 0/64 (PE stationary base constraint)
            st_tiles = [ustore.tile([128, R], BF16, tag=f"u{j}", name=f"u{j}")
                        for j in range(NU)]

            def slab(s):
                return st_tiles[s // 2], 64 * (s % 2)

            vtz_ps = vtzps.tile([KATT, KATT], F32, tag="vtz")

            # ---------------- phase 1 ----------------
            with tc.tile_pool(name="p1x", bufs=5) as p1x, \
                 tc.tile_pool(name="p1xt", bufs=3) as p1xt, \
                 tc.tile_pool(name="p1vzf", bufs=4) as p1vzf, \
                 tc.tile_pool(name="p1vz", bufs=3) as p1vz, \
                 tc.tile_pool(name="p1cs", bufs=4) as p1cs, \
                 tc.tile_pool(name="xtps", bufs=2, space="PSUM") as xtps, \
                 tc.tile_pool(name="mmps", bufs=2, space="PSUM") as mmps, \
                 tc.tile_pool(name="scps", bufs=1, space="PSUM") as scps:

                vz_tiles = [p1vz.tile([128, 2, G, 2 * KATT], BF16,
                                      tag="vzsb", name=f"vzsb{j}")
                            for j in range(3)]

                st_pair = {}
                st_xtps = {}    # transp out PSUM, stage 0 -> -1
                st_xtsb = {}    # x^T SBUF, stage -1 -> -2
                st_mm = {}      # GEMM out PSUM, stage -2 -> -3
                st_relu = {}    # X^T bf16, stage -3 -> -4/-5 (park+stageC)
                st_scout = {}   # stageC PSUM (tile, q), even iters
                st_vzrows = {}  # V|Z rows SBUF (tile, q)
                n_vtz = 0

                # ---- tail first (36 rows zero-padded, even layout) so
                # its serial chain hides in the pipeline fill and its
                # VtZ opens the accumulation group ----
                x_tl = p1x.tile([128, 2, G, D_IN], F32R, tag="xin")
                zpad = small.tile([128, D_IN], F32, tag="zpad", name="zpad")
                nc.gpsimd.memset(zpad[:], 0.0)
                nc.vector.tensor_copy(x_tl[:, 0, 0], zpad[:])
                nc.sync.dma_start(x_tl[0:TAIL, 0, 0, :],
                                  x_in[NS * R:NR, :])
                xt_ps = xtps.tile([128, 2, R], F32R, tag="xt")
                for k in range(2):
                    nc.tensor.transpose(
                        xt_ps[:, k, 0:128],
                        x_tl[:, 0, 0, k * 128:(k + 1) * 128], identr[:])
                xt_sb = p1xt.tile([128, 2, R], BF16, tag="xtsb")
                nc.vector.tensor_copy(xt_sb[:, :, 0:128], xt_ps[:, :, 0:128])
                mm_ps = mmps.tile([128, R], F32, tag="xmm")
                nc.tensor.matmul(mm_ps[:, 0:128], w_r[:, 0, 0],
                                 xt_sb[:, 0, 0:128], start=True, stop=False)
                nc.tensor.matmul(mm_ps[:, 0:128], w_r[:, 0, 1],
                                 xt_sb[:, 1, 0:128], start=False, stop=True)
                ut_t, _ = slab(NS)
                cs_t = p1cs.tile([128, 1], F32, tag="cst")
                vzf_t = p1vzf.tile([128, R], BF16, tag="vzf")
                nc.scalar.activation(vzf_t[:, 0:128], mm_ps[:, 0:128],
                                     mybir.ActivationFunctionType.Relu,
                                     bias=b_sb[:, 0:1],
                                     accum_out=cs_t[:])
                nc.gpsimd.tensor_add(csum_e[:], csum_e[:], cs_t[:])
                nc.vector.tensor_copy(ut_t[0:64, 0:128], vzf_t[0:64, 0:128])
                sc_tl = scps.tile([128, 2, G, 2 * KATT], BF16, tag="sc")
                nc.tensor.transpose(sc_tl[:, 0, 0, :], vzf_t[64:128, 0:128],
                                    identb[64:128, 64:128])
                vz_tl = vz_tiles[2]
                nc.vector.tensor_copy(vz_tl[:, 0, 0, 0:2 * KATT],
                                      sc_tl[:, 0, 0, :])
                nc.tensor.matmul(vtz_ps[:], vz_tl[:, 0, 0, 0:KATT],
                                 vz_tl[:, 0, 0, KATT:2 * KATT],
                                 start=True, stop=False)
                n_vtz += 1

                # ---- steady-state pipeline ----
                for p in range(2):
                    xp = p1x.tile([128, 2, G, D_IN], F32R, tag="xin")
                    eng = nc.sync if p % 2 == 0 else nc.gpsimd
                    eng.dma_start(
                        xp[:],
                        x_in[2 * p * R:(2 * p + 2) * R, :].rearrange(
                            "(t p g) d -> p t g d", t=2, g=G))
                    st_pair[p] = xp

                for i in range(NS + 9):
                    # oldest PE work first so a late x DMA can't block
                    # ready matmuls in the in-order PE queue
                    if True:
                        # stageC (bf16 transpose) before VtZ (bf16 matmul)
                        # so VtZ sits adjacent to the bf16 GEMM: one fewer
                        # PE mode-switch drain per iteration
                        sc_ps = None
                        for q, s in enumerate((i - 4,)):
                            if not 0 <= s < NS:
                                continue
                            if sc_ps is None:
                                sc_ps = scps.tile([128, 2, G, 2 * KATT],
                                                  BF16, tag="sc")
                            vzf = st_relu[s]
                            lo = 64 * ((s + 1) % 2)
                            for c in range(G):
                                nc.tensor.transpose(
                                    sc_ps[:, q, c, :],
                                    vzf[lo:lo + 64, c * 128:(c + 1) * 128],
                                    identb[lo:lo + 64, lo:lo + 64])
                            st_scout[s] = (sc_ps, q)
                        for s in (i - 7,):
                            if not 0 <= s < NS:
                                continue
                            vzb, q = st_vzrows.pop(s)
                            for c in range(G):
                                nc.tensor.matmul(
                                    vtz_ps[:], vzb[:, q, c, 0:KATT],
                                    vzb[:, q, c, KATT:2 * KATT],
                                    start=False,
                                    stop=(n_vtz == NS) and (c == G - 1))
                            n_vtz += 1

                    if 0 <= i - 2 < NS:
                        s = i - 2
                        xt_sb = st_xtsb.pop(s)
                        par = s % 2
                        mm_ps = mmps.tile([128, R], F32, tag="xmm")
                        nc.tensor.matmul(mm_ps[:], w_r[:, par, 0],
                                         xt_sb[:, 0, :], start=True,
                                         stop=False)
                        nc.tensor.matmul(mm_ps[:], w_r[:, par, 1],
                                         xt_sb[:, 1, :], start=False,
                                         stop=True)
                        st_mm[s] = mm_ps

                    if i < NS:
                        s = i
                        if s % 2 == 0:
                            p = s // 2 + 2
                            if p < NP:
                                xp = p1x.tile([128, 2, G, D_IN], F32R,
                                              tag="xin")
                                eng = nc.sync if p % 2 == 0 else nc.gpsimd
                                eng.dma_start(
                                    xp[:],
                                    x_in[2 * p * R:(2 * p + 2) * R, :]
                                    .rearrange("(t p g) d -> p t g d",
                                               t=2, g=G))
                                st_pair[p] = xp
                        xp = st_pair[s // 2]
                        xt_ps = xtps.tile([128, 2, R], F32R, tag="xt")
                        for k in range(2):
                            for g in range(G):
                                nc.tensor.transpose(
                                    xt_ps[:, k, g * 128:(g + 1) * 128],
                                    xp[:, s % 2, g,
                                       k * 128:(k + 1) * 128],
                                    identr[:])
                        st_xtps[s] = xt_ps
                        if s % 2 == 1:
                            st_pair.pop(s // 2)

                    # DVE: x^T copy first (its consumer, the GEMM, is
                    # the tightest dependency)
                    if 0 <= i - 1 < NS:
                        s = i - 1
                        xt_ps = st_xtps.pop(s)
                        xt_sb = p1xt.tile([128, 2, R], BF16, tag="xtsb")
                        h = XSPL // 2
                        nc.vector.tensor_copy(xt_sb[:, :, 0:h],
                                              xt_ps[:, :, 0:h])
                        nc.scalar.copy(xt_sb[:, :, h:R], xt_ps[:, :, h:R])
                        st_xtsb[s] = xt_sb

                    if True:
                        for s in (i - 5,):
                            if s in st_scout:
                                sc_ps, q = st_scout.pop(s)
                                vzb = vz_tiles[s % 3]
                                nc.vector.tensor_copy(vzb[:, q],
                                                      sc_ps[:, q])
                                st_vzrows[s] = (vzb, q)

                    if 0 <= i - 4 < NS:
                        s = i - 4
                        vzf = st_relu[s]
                        uo = 64 * (s % 2)
                        ut_t, _ = slab(s)
                        nc.vector.tensor_copy(ut_t[uo:uo + 64, :],
                                              vzf[uo:uo + 64, :])
                    st_relu.pop(i - 6, None)

                    if 0 <= i - 3 < NS:
                        s = i - 3
                        mm_ps = st_mm.pop(s)
                        par = s % 2
                        vzf = p1vzf.tile([128, R], BF16, tag="vzf")
                        cs_t = p1cs.tile([128, 1], F32, tag="cst")
                        nc.scalar.activation(
                            vzf[:], mm_ps[:],
                            mybir.ActivationFunctionType.Relu,
                            bias=b_sb[:, par:par + 1],
                            accum_out=cs_t[:])
                        csacc = csum_e if par == 0 else csum_o
                        nc.gpsimd.tensor_add(csacc[:], csacc[:], cs_t[:])
                        st_relu[s] = vzf

            # ---------------- phase 2 pools + T prep ----------------
            with tc.tile_pool(name="p2o", bufs=NPREP + 6) as p2o, \
                 tc.tile_pool(name="p2ps", bufs=3, space="PSUM") as p2ps, \
                 tc.tile_pool(name="p2tps", bufs=2, space="PSUM") as p2tps:

                def t_prep(s, ot, eng):
                    # T rows for supertile s -> ot[:, :, 32:64]. The full
                    # 64-row [U^T;T^T] slab is transposed so the PE
                    # stationary base stays 0/64; U half unused.
                    t2 = p2tps.tile([128, G, 2 * KATT], BF16, tag="t2")
                    tt, to = slab(s)
                    for c in range(G):
                        nc.tensor.transpose(
                            t2[:, c, :],
                            tt[to:to + 64, c * 128:(c + 1) * 128],
                            identb[to:to + 64, to:to + 64])
                    if eng == 0:
                        nc.vector.tensor_copy(ot[:, :, KATT:2 * KATT],
                                              t2[:, :, KATT:2 * KATT])
                    else:
                        nc.scalar.copy(ot[:, :, KATT:2 * KATT],
                                       t2[:, :, KATT:2 * KATT])

                ots = []
                for s in range(min(NPREP, NS)):
                    ot = p2o.tile([128, G, 2 * KATT], F32, tag="oo")
                    t_prep(s, ot, s % 2)
                    ots.append(ot)

                # ---------------- stats + collective ----------------
                vtz_sb = small.tile([KATT, KATT], F32, tag="vtzsb")
                nc.vector.tensor_sub(vtz_sb[:], vtz_ps[:], cvz_sb[:])
                nc.vector.tensor_sub(csum_e[:], csum_e[:], cut_sb[:])
                nc.sync.dma_start(
                    stats_in[0:1024].rearrange("(p q) -> p q", p=KATT),
                    vtz_sb[:])
                # colsum(V) = even rows 64:96 + odd rows 0:32;
                # colsum(U) = even rows 0:32 + odd rows 64:96
                cs_fold = small.tile([KATT, 2], F32, tag="csfold")
                nc.gpsimd.tensor_copy(cs_fold[:, 0:1], csum_e[64:96, :])
                nc.gpsimd.tensor_copy(cs_fold[:, 1:2], csum_o[64:96, :])
                nc.vector.tensor_add(cs_fold[:, 0:1], cs_fold[:, 0:1],
                                     csum_o[0:32, :])
                nc.vector.tensor_add(cs_fold[:, 1:2], cs_fold[:, 1:2],
                                     csum_e[0:32, :])
                nc.sync.dma_start(
                    stats_in[1024:1088].rearrange("(q p) -> p q", q=2),
                    cs_fold[:])
                nc.gpsimd.collective_compute(
                    "AllReduce", mybir.AluOpType.add,
                    replica_groups=[list(range(N_CORES))],
                    ins=[stats_in.ap().opt()], outs=[stats_out.ap().opt()])
                us_all = small.tile([KATT, 2], F32, tag="usall")
                nc.sync.dma_start(
                    us_all[:],
                    stats_out[1024:1088].rearrange("(q p) -> p q", q=2))
                vtz_g = small.tile([128, KATT], F32, tag="vtzg")
                for q in range(2):
                    nc.sync.dma_start(
                        vtz_g[64 * q:64 * q + KATT, :],
                        stats_out[0:1024].rearrange("(p q) -> p q", p=KATT))

                with tc.tile_pool(name="dps", bufs=1, space="PSUM") as dps:
                    us_ps = dps.tile([1, 1], F32, tag="us")
                    nc.tensor.matmul(us_ps[:], us_all[:, 1:2],
                                     us_all[:, 0:1], start=True, stop=True)
                    nf_sb = small.tile([1, 1], F32, tag="nf")
                    nc.scalar.activation(nf_sb[:], us_ps[:],
                                         mybir.ActivationFunctionType.Copy,
                                         bias=EPS, scale=1.0 / N_TOTAL)
                    d_sb = small.tile([1, 1], F32, tag="dsb")
                    nc.vector.reciprocal(d_sb[:], nf_sb[:])
                    d_ps = dps.tile([128, 1], F32, tag="dps")
                    nc.tensor.matmul(d_ps[:], ones1[:], d_sb[:],
                                     start=True, stop=True)
                    d_all = small.tile([128, 1], F32, tag="dall")
                    nc.vector.tensor_copy(d_all[:], d_ps[:])
                vtz_d = small.tile([128, KATT], F32, tag="vtzd")
                m_all = small.tile([128, KATT], BF16, tag="mall")
                for q in (0, 64):
                    nc.vector.tensor_scalar_mul(vtz_d[q:q + KATT, :],
                                                vtz_g[q:q + KATT, :],
                                                d_all[q:q + KATT, :])
                    nc.vector.tensor_copy(m_all[q:q + KATT, :],
                                          vtz_d[q:q + KATT, :])

                # tail (full-width ops on the zero-padded block)
                ot = p2o.tile([128, G, 2 * KATT], F32, tag="oo")
                t2 = p2tps.tile([128, G, 2 * KATT], BF16, tag="t2")
                tt, to = slab(NS)
                nc.tensor.transpose(t2[:, 0, :], tt[to:to + 64, 0:128],
                                    identb[to:to + 64, to:to + 64])
                nc.scalar.copy(ot[:, 0, KATT:2 * KATT],
                               t2[:, 0, KATT:2 * KATT])
                res_ps = p2ps.tile([128, G, KATT], F32, tag="res")
                nc.tensor.matmul(res_ps[:, 0, :],
                                 tt[to:to + 32, 0:128],
                                 m_all[to:to + 32, :], start=True, stop=True)
                nc.vector.tensor_copy(ot[:, 0, 0:KATT], res_ps[:, 0, :])
                nc.sync.dma_start(out_d[NS * R:NR, :], ot[0:TAIL, 0, :])

                # T-prep runs 2 tiles ahead of res so its PE transposes
                # and PSUM->SBUF copies never stall the res/DMA chain
                ot_by_s = dict(enumerate(ots))
                for s in range(NS):
                    sp = s + 2
                    if len(ots) <= sp < NS:
                        otp = p2o.tile([128, G, 2 * KATT], F32, tag="oo")
                        t_prep(sp, otp, 0 if sp % 3 != 0 else 1)
                        ot_by_s[sp] = otp
                    ot = ot_by_s.pop(s)
                    res_ps = p2ps.tile([128, G, KATT], F32, tag="res")
                    ut_t, uo = slab(s)
                    for c in range(G):
                        nc.tensor.matmul(
                            res_ps[:, c, :],
                            ut_t[uo:uo + 32, c * 128:(c + 1) * 128],
                            m_all[uo:uo + 32, :],
                            start=True, stop=True)
                    nc.vector.tensor_copy(ot[:, :, 0:KATT], res_ps[:])
                    dma_eng = (nc.sync, nc.scalar, nc.gpsimd)[s % 3]
                    dma_eng.dma_start(
                        out_d[s * R:(s + 1) * R, :].rearrange(
                            "(p g) q -> p g q", g=G), ot[:])
    nc.compile()
    return nc


def _prep_inputs(x, W, b):
    W = np.asarray(W, dtype=np.float32)
    b = np.asarray(b, dtype=np.float32)
    we = np.ascontiguousarray(W[:, PERM_E])
    wo = np.ascontiguousarray(W[:, PERM_O])
    be = np.ascontiguousarray(b[PERM_E])
    bo = np.ascontiguousarray(b[PERM_O])
    rb = np.maximum(be, 0.0).astype(np.float32)
    n_pad = 128 - TAIL
    # tail pollution: relu(b) in the padded rows (even layout)
    rbv, rbz = rb[64:96], rb[96:128]
    corrvz = (n_pad * np.outer(rbv, rbz)).astype(np.float32)
    corrcs = (n_pad * rb).astype(np.float32)
    x = np.asarray(x, dtype=np.float32)
    in_maps = []
    for c in range(N_CORES):
        in_maps.append({
            "xin": np.ascontiguousarray(x[c * NR:(c + 1) * NR]),
            "we": we, "wo": wo, "be": be, "bo": bo,
            "corrvz": corrvz, "corrcs": corrcs,
        })
    return in_maps


def _run(x, W, b, trace=False):
    if "nc" not in _CACHE:
        _CACHE["nc"] = _build()
    nc = _CACHE["nc"]
    in_maps = _prep_inputs(x, W, b)
    res = run_bass_kernel_spmd(nc, in_maps, core_ids=list(range(N_CORES)),
                               trace=trace)
    out = np.concatenate([r["out"] for r in res.results], axis=0)
    return out, res


def kernel(x, W, b):
    out, _ = _run(x, W, b, trace=False)
    return out
